# revision 23
# baseline (speedup 1.0000x reference)
"""Spatio-temporal Hawkes process log-likelihood on Trainium2 (Bass/Tile).

Computes, for x[B, L, 3] = (t, s1, s2) and scalars mu/alpha/beta/sigma:
  lams[b, i]  = softplus(sum_{j<i} K(x_i, x_j) * 1[t_j>0] + mu) + 1e-5
  loglik[b]   = sum_i log(lams[b,i]) * 1[t_i>0]
              - UNIT_VOL * sum_{r, g} softplus(sum_j K((tt_r, ss_g), x_j) * m + mu)
with K(x, y) = norm * exp(-beta*(t_x - t_y) - |s_x - s_y|^2 / (2 sigma^2)),
norm = alpha*beta/(2 pi sigma^2), over a 50 x 50 x 50 (t, s1, s2) grid.

Strategy (one batch element per NeuronCore, 8 cores, data-parallel):
  The grid kernel factorizes in time AND is separable in space:
    K((tt_r, ss_g), x_j) = [norm * 1[0<t_j<=tt_r] * e^{beta(t_j-tt_r)}]
                         * e^{-inv2sig2 (g1_i-s1_j)^2} * e^{-inv2sig2 (g2_k-s2_j)^2}
  so per core we build two tiny per-axis tables A[j,i] (25 cols/half)
  and B[j,k] (50 cols) via K<=4 quadratic-expansion matmuls + ACT exp,
  expand G[j,g] = A*B with one broadcast-AP multiply, build the
  temporal weight matrix W[j,r], and get the softplus argument as
  W.T @ G on the PE (bf16 operands, fp32 PSUM accumulation).
  softplus+row-sum is chunk-pipelined against the matmuls, with the
  row sums riding the accum_out ports of the DVE relu and ACT log1p.
  The per-event [L, L] exponent is built by 2 accumulated K=2 fp32
  matmuls over pair-packed rank-1 factors (fp32 because the quadratic
  expansion cancels catastrophically in low precision), one ACT exp,
  and a masked row-reduce fused into a scalar_tensor_tensor.

  Partition packing: the 2500 spatial grid points are split in two
  halves of 1250; partitions 0:64 hold events-vs-half0, 64:128 hold
  events-vs-half1, so elementwise engines run at full 128-lane width.

softplus is decomposed as relu(v) + log1p(exp(-|v|)) and the
activation-table map is patched during compile so every ACT func
(Exp/Ln/Abs/Copy) resolves to the single `natural_log_exp_and_others`
set -> one table load, no set thrashing.

All tiny per-core staging (dup columns, concatenated rows, packed
rank-1 factors) is marshalled host-side as pure copies - engines can
only address SBUF partition starts of 0/32/64/96, so single-row writes
at other partitions are not expressible on-device.  Emission order is
tuned so no in-order engine stream blocks on a late dependency.
"""

import math
import numpy as np
from contextlib import ExitStack

R = 50                      # INT_RES (time and each spatial axis)
RG = R * R                  # 2500 spatial grid points
HALF = RG // 2              # 1250
NCORES = 8
UNIT_VOL = 1.0 / float(R ** 3)
BIG_NEG = 1.0e30
CHUNKS = ((0, 512), (512, 512), (1024, HALF - 1024))

_prog_cache: dict = {}


def _const_arrays(L: int, norm: float, beta: float, inv2sig2: float):
    f32 = np.float32
    g1 = np.linspace(0.0, 1.0, R).astype(f32)
    g2 = np.linspace(0.0, 1.0, R).astype(f32)

    ccols = np.zeros((128, 2), f32)
    ccols[:, 0] = 1.0                                   # ones column
    ccols[:, 1] = (np.arange(128) % 64 < R)             # sel (valid r rows)

    ctril = (norm * np.tril(np.ones((L, L), np.float64), -1)).astype(f32)

    # crows [4, 256]:
    #   [0:1, 0:128]   ones row
    #   [0:1, 128:178] -linspace(0,1,R)
    #   [0:2, 178:180] per-event STT coefficient columns
    #   [0:6, 180:255] rhs of combined A|B table matmul (block diagonal):
    #       cols 0:25  rows [g1lo^2; g1hi^2; -2g1lo; -2g1hi; 0; 0]
    #       cols 25:75 rows [0; 0; 0; 0; g2^2; -2g2]
    crows = np.zeros((6, 256), f32)
    crows[0, 0:128] = 1.0
    crows[0, 128:178] = -np.linspace(0.0, 1.0, R)
    crows[0, 178] = -beta; crows[1, 178] = 2.0 * inv2sig2    # colA -> [u; a1]
    crows[0, 179] = beta;  crows[1, 179] = 2.0 * inv2sig2    # colB -> [v; a2]
    g1lo, g1hi = g1[0:25], g1[25:50]
    crows[0, 180:205] = g1lo ** 2
    crows[1, 180:205] = g1hi ** 2
    crows[2, 180:205] = -2.0 * g1lo
    crows[3, 180:205] = -2.0 * g1hi
    crows[4, 205:255] = g2 ** 2
    crows[5, 205:255] = -2.0 * g2
    return dict(ccols=ccols, ctril=ctril, crows=crows)


def _marshal_core_inputs(t, s1, s2):
    """Pure-layout staging of one sequence's inputs (no arithmetic).

    icols [128, 3]: t/s1/s2 duplicated into both partition halves.
    irows [8, 704]:
      [0:1, 0:192]   t | s1 | s2 concatenated rows
      [0:4, 192:320] lhsT of the A-table matmul
                     [ind_lo; ind_hi; s1*ind_lo; s1*ind_hi]
      [0:2, 320:384] [t; s1]   (pair-packed per-event row inputs)
      [0:2, 384:448] [t; s2]
      [0:2, 448:512] [ones; s1] (rhs of per-event matmul 1)
      [0:2, 512:576] [ones; s2] (lhsT of per-event matmul 2)
      [0:6, 192:320] K=6 lhsT of the combined A|B table matmul
                     [ind_lo; ind_hi; s1*ind_lo; s1*ind_hi; ones128; s2_dup]
    """
    f32 = np.float32
    L = t.shape[0]
    icols = np.zeros((128, 3), f32)
    icols[0:L, 0] = t; icols[64:64 + L, 0] = t
    icols[0:L, 1] = s1; icols[64:64 + L, 1] = s1
    icols[0:L, 2] = s2; icols[64:64 + L, 2] = s2
    irows = np.zeros((8, 704), f32)
    irows[0, 0:L] = t
    irows[0, L:2 * L] = s1
    irows[0, 2 * L:3 * L] = s2
    irows[0, 192:256] = 1.0                       # ind_lo
    irows[1, 256:320] = 1.0                       # ind_hi
    irows[2, 192:192 + L] = s1
    irows[3, 256:256 + L] = s1
    irows[0, 320:320 + L] = t;   irows[1, 320:320 + L] = s1
    irows[0, 384:384 + L] = t;   irows[1, 384:384 + L] = s2
    irows[0, 448:448 + L] = 1.0; irows[1, 448:448 + L] = s1
    irows[0, 512:512 + L] = 1.0; irows[1, 512:512 + L] = s2
    irows[4, 192:320] = 1.0
    irows[5, 192:192 + L] = s2;  irows[5, 256:256 + L] = s2
    return {"icols": icols, "irows": irows}


def _patched_act_tables(orig_fn, preferred="natural_log_exp_and_others"):
    """Wrap get_activation_tables so every function present in the
    preferred set resolves only to it (same names/order, so the emitted
    act_func_set_id still indexes the real act_info.json)."""
    import functools

    @functools.cache
    def wrapper(arch):
        tables = dict(orig_fn(arch))
        pref = tables.get(preferred)
        if not pref:
            return tables
        return {
            name: (funcs if name == preferred else funcs - pref)
            for name, funcs in tables.items()
        }
    return wrapper


def _build_program(mu: float, beta: float, inv2sig2: float, norm: float, L: int):
    import concourse.bass as bass
    import concourse.bacc as bacc
    import concourse.tile as tile
    import concourse.mybir as mybir

    f32 = mybir.dt.float32
    bf16 = mybir.dt.bfloat16
    Act = mybir.ActivationFunctionType
    Op = mybir.AluOpType

    nc = bacc.Bacc("TRN2", target_bir_lowering=False, debug=False,
                   enable_asserts=True, num_devices=NCORES)

    # ---- DRAM I/O
    icols_d = nc.dram_tensor("icols", [128, 3], f32, kind="ExternalInput").ap()
    irows_d = nc.dram_tensor("irows", [8, 704], f32, kind="ExternalInput").ap()
    ccols_d = nc.dram_tensor("ccols", [128, 2], f32, kind="ExternalInput").ap()
    ctril_d = nc.dram_tensor("ctril", [L, L], f32, kind="ExternalInput").ap()
    crows_d = nc.dram_tensor("crows", [6, 256], f32, kind="ExternalInput").ap()
    lams_o = nc.dram_tensor("lams_o", [L], f32, kind="ExternalOutput").ap()
    ll_o = nc.dram_tensor("ll_o", [1], f32, kind="ExternalOutput").ap()

    with tile.TileContext(nc) as tc, ExitStack() as ctx:
        pool = ctx.enter_context(tc.tile_pool(name="sbuf", bufs=1))
        cpool = ctx.enter_context(tc.tile_pool(name="chunk", bufs=2))
        psum = ctx.enter_context(tc.tile_pool(name="psum", bufs=1,
                                              space=bass.MemorySpace.PSUM))
        psmall = ctx.enter_context(tc.tile_pool(name="psmall", bufs=3,
                                                space=bass.MemorySpace.PSUM))

        # ---- loads (5 small DMAs on 3 queues)
        irows = pool.tile([8, 704], f32)
        nc.sync.dma_start(irows[:], irows_d[:])
        icols = pool.tile([128, 3], f32)
        nc.sync.dma_start(icols[:], icols_d[:])
        crows = pool.tile([6, 256], f32)
        nc.gpsimd.dma_start(crows[:], crows_d[:])
        ctril = pool.tile([L, L], f32)
        nc.gpsimd.dma_start(ctril[:], ctril_d[:])
        ccols = pool.tile([128, 2], f32)
        nc.scalar.dma_start(ccols[:], ccols_d[:])

        t_col = icols[:, 0:1]
        s1_col = icols[:, 1:2]
        s2_col = icols[:, 2:3]
        t_row = irows[0:1, 0:L]
        s1_row = irows[0:1, L:2 * L]
        s2_row = irows[0:1, 2 * L:3 * L]
        lhsT6AB = irows[0:6, 192:320]
        pA_in = irows[0:2, 320:320 + L]
        pB_in = irows[0:2, 384:384 + L]
        rhs_mm1 = irows[0:2, 448:448 + L]
        lhsT_mm2 = irows[0:2, 512:512 + L]

        ones_col = ccols[:, 0:1]
        sel_col = ccols[:, 1:2]
        ones_r = crows[0:1, 0:128]
        ones_r64 = crows[0:1, 0:L]
        negttg = crows[0:1, 128:178]
        scoefA = crows[0:2, 178:179]
        scoefB = crows[0:2, 179:180]
        rhsAB = crows[0:6, 180:255]

        negmu_col = pool.tile([128, 1], f32)
        nc.vector.memset(negmu_col[:], -mu)

        # ---- per-partition spatial biases: -inv2sig2*s1^2, -inv2sig2*s2^2
        biasA = pool.tile([128, 1], f32)
        nc.vector.tensor_scalar(biasA[:], s1_col, s1_col, -inv2sig2,
                                Op.mult, Op.mult)
        biasB = pool.tile([128, 1], f32)
        nc.vector.tensor_scalar(biasB[:], s2_col, s2_col, -inv2sig2,
                                Op.mult, Op.mult)

        # ---- temporal weights W_T[j(packed), r] (norm folded into mask)
        bc_ps = psmall.tile([128, R], f32, tag="small")
        nc.tensor.matmul(bc_ps[:], ones_r, negttg, start=True, stop=True)
        dtW = pool.tile([128, R], f32)
        nc.vector.tensor_scalar(dtW[:], bc_ps[:], t_col, None, Op.add)
        Ew = pool.tile([128, R], f32)
        nc.scalar.activation(Ew[:], dtW[:], Act.Exp, scale=beta)
        hn_col = pool.tile([128, 1], f32)
        nc.vector.tensor_scalar(hn_col[:], t_col, 0.0, norm, Op.is_gt, Op.mult)
        h_col = pool.tile([128, 1], f32)
        nc.vector.tensor_scalar(h_col[:], t_col, 0.0, None, Op.is_gt)
        Mw = pool.tile([128, R], f32)
        nc.vector.tensor_scalar(Mw[:], dtW[:], 0.0, hn_col[:, 0:1], Op.is_le, Op.mult)
        WT2 = pool.tile([128, 64], bf16)
        nc.vector.memset(WT2[:], 0.0)
        nc.vector.tensor_tensor(WT2[:, 0:R], Ew[:], Mw[:], Op.mult)

        # ---- separable spatial kernel: G[p, i*50+k] = A[p, i] * B[p, k]
        psAB = psmall.tile([128, 80], f32, tag="small")
        nc.tensor.matmul(psAB[:, 0:75], lhsT6AB, rhsAB, start=True, stop=True)
        A2 = pool.tile([128, 25], f32)
        nc.scalar.activation(A2[:], psAB[:, 0:25], Act.Exp,
                             scale=-inv2sig2, bias=biasA[:, 0:1])
        B2 = pool.tile([128, R], f32)
        nc.scalar.activation(B2[:], psAB[:, 25:75], Act.Exp,
                             scale=-inv2sig2, bias=biasB[:, 0:1])
        G = pool.tile([128, HALF], bf16)

        def _emit_g(goff, gw):
            a0 = goff // R
            nc.vector.tensor_tensor(
                G[:, goff:goff + gw].rearrange("p (a b) -> p a b", a=gw // R),
                A2[:, a0:a0 + gw // R].unsqueeze(2).broadcast_to(
                    [128, gw // R, R]),
                B2[:].unsqueeze(1).broadcast_to([128, gw // R, R]),
                Op.mult)

        _emit_g(0, 600)

        # ---- per-event exponent via 2 accumulated K=2 matmuls (fp32)
        sq1 = pool.tile([1, L], f32)
        nc.vector.tensor_tensor(sq1[:], s1_row, s1_row, Op.mult)
        sq2 = pool.tile([1, L], f32)
        nc.vector.tensor_tensor(sq2[:], s2_row, s2_row, Op.mult)
        ssum = pool.tile([1, L], f32)
        nc.vector.tensor_tensor(ssum[:], sq1[:], sq2[:], Op.add)
        w2 = pool.tile([2, L], f32)
        nc.vector.memset(w2[:], 0.0)
        nc.vector.tensor_scalar(w2[0:1, :], ssum[:], -inv2sig2, None, Op.mult)

        pairA = pool.tile([2, L], f32)
        nc.vector.scalar_tensor_tensor(pairA[:], pA_in, scoefA, w2[:],
                                       Op.mult, Op.add)
        pairB = pool.tile([2, L], f32)
        nc.vector.scalar_tensor_tensor(pairB[:], pB_in, scoefB, w2[:],
                                       Op.mult, Op.add)
        ha = pool.tile([1, L], f32)
        nc.vector.tensor_scalar(ha[:], t_row, 0.0, 1.0, Op.is_gt, Op.subtract)
        hm = pool.tile([1, L], f32)
        nc.vector.tensor_scalar(hm[:], ha[:], BIG_NEG, None, Op.mult)
        nc.vector.tensor_tensor(pairB[0:1, :], pairB[0:1, :], hm[:], Op.add)

        zev_ps = psmall.tile([L, L], f32, tag="small")
        nc.tensor.matmul(zev_ps[:], pairA[:], rhs_mm1, start=True, stop=False)
        nc.tensor.matmul(zev_ps[:], lhsT_mm2, pairB[:], start=False, stop=True)

        Ke = pool.tile([L, L], f32)
        nc.scalar.activation(Ke[:], zev_ps[:], Act.Exp)
        Km = pool.tile([L, L], f32)
        lam_col = pool.tile([L, 1], f32)
        nc.vector.scalar_tensor_tensor(Km[:], Ke[:], 0.0, ctril[:],
                                       Op.add, Op.mult, accum_out=lam_col[:])
        _emit_g(600, 650)
        # row sums of G (for the analytic relu part of softplus):
        # Gsum[p] = sum_i A2[p,i] * sum_k B2[p,k]
        sA = pool.tile([128, 1], f32)
        nc.vector.tensor_reduce(sA[:], A2[:], mybir.AxisListType.X, Op.add)
        sB = pool.tile([128, 1], f32)
        nc.vector.tensor_reduce(sB[:], B2[:], mybir.AxisListType.X, Op.add)
        gsum = pool.tile([128, 1], bf16)
        nc.vector.tensor_tensor(gsum[:], sA[:], sB[:], Op.mult)

        # lams = softplus(lam_raw + mu) + 1e-5; lam_raw >= 0 and mu >= 0, so
        # softplus(v) = v + log1p(exp(-v)) with no abs/relu needed
        ee = pool.tile([L, 1], f32)
        nc.scalar.activation(ee[:], lam_col[:], Act.Exp, scale=-1.0,
                             bias=negmu_col[0:L, 0:1])
        lp = pool.tile([L, 1], f32)
        nc.scalar.activation(lp[:], ee[:], Act.Ln, bias=1.0)
        lams_col = pool.tile([L, 1], f32)
        nc.vector.scalar_tensor_tensor(lams_col[:], lam_col[:], mu + 1e-5,
                                       lp[:], Op.add, Op.add)
        nc.sync.dma_start(lams_o.unsqueeze(1), lams_col[:])
        lnl = pool.tile([L, 1], f32)
        nc.scalar.activation(lnl[:], lams_col[:], Act.Ln)
        lnm = pool.tile([L, 1], f32)
        nc.vector.tensor_tensor(lnm[:], lnl[:], h_col[0:L, 0:1], Op.mult)

        # ---- grid: matmuls per 512 PSUM bank; softplus pipelined in two
        #      625-wide chunks (bank-level deps let them overlap the mms).
        # v = z + mu >= 0 always (z is a sum of nonnegative products and
        # mu >= 0), so softplus(v) = v + log1p(exp(-v)); the linear part
        # is summed analytically via Gsum below, only log1p runs per-element.
        # acc cols 0:3 = per-chunk log1p row sums
        acc = pool.tile([128, 3], f32)
        zz = pool.tile([128, 512], f32)
        nc.vector.memset(zz[:], 0.0)
        z_ps = psum.tile([128, 1536], f32, tag="z")
        SP_CHUNKS = CHUNKS
        emitted = set()

        def _mms_for(lo, hi):
            for moff, mw in CHUNKS:
                if moff in emitted or moff >= hi or moff + mw <= lo:
                    continue
                emitted.add(moff)
                for h in (0, 1):
                    p0 = h * 64
                    nc.tensor.matmul(z_ps[p0:p0 + 64, moff:moff + mw],
                                     WT2[p0:p0 + 64, 0:64],
                                     G[p0:p0 + 64, moff:moff + mw],
                                     start=True, stop=True)

        for ci, (off, w) in enumerate(SP_CHUNKS):
            _mms_for(off, off + w)
            e_c = cpool.tile([128, 640], f32, tag="e_c")
            nc.scalar.activation(e_c[:, 0:w], z_ps[:, off:off + w], Act.Exp,
                                 scale=-1.0, bias=negmu_col[:, 0:1])
            l_c = cpool.tile([128, 640], f32, tag="l_c")
            nc.scalar.activation(l_c[:, 0:w], e_c[:, 0:w], Act.Ln, bias=1.0)
            s_c = cpool.tile([128, 640], f32, tag="s_c")
            nc.vector.scalar_tensor_tensor(s_c[:, 0:w], l_c[:, 0:w], 0.0,
                                           zz[:, 0:w], Op.add, Op.add,
                                           accum_out=acc[:, ci:ci + 1])

        # ---- reductions and final combine
        # zsum[r] = sum_g z[r, g] via Gsum (both packed halves accumulate)
        zs_ps = psmall.tile([64, 1], f32, tag="small")
        nc.tensor.matmul(zs_ps[:], WT2[0:64, 0:64], gsum[0:64, 0:1],
                         start=True, stop=False)
        nc.tensor.matmul(zs_ps[:], WT2[64:128, 0:64], gsum[64:128, 0:1],
                         start=False, stop=True)
        zc = pool.tile([64, 1], f32)
        nc.vector.tensor_copy(zc[:], zs_ps[:])

        red_ps = psmall.tile([1, 8], f32, tag="small")
        nc.tensor.matmul(red_ps[0:1, 0:1], lnm[:, 0:1], ones_col[0:L, 0:1],
                         start=True, stop=True)
        nc.tensor.matmul(red_ps[0:1, 1:2], zc[:, 0:1], ones_col[0:L, 0:1],
                         start=True, stop=True)
        nc.tensor.matmul(red_ps[0:1, 2:5], sel_col, acc[:, 0:3],
                         start=True, stop=True)

        # loglik = sumlog - V*(zsum_tot + 125000*mu + l1p_tot)
        ird = pool.tile([1, 1], f32)
        nc.vector.tensor_reduce(ird[:], red_ps[0:1, 1:5],
                                mybir.AxisListType.X, Op.add)
        sl_sb = pool.tile([1, 1], f32)
        nc.vector.tensor_scalar(sl_sb[:], red_ps[0:1, 0:1],
                                -UNIT_VOL * float(RG * R) * mu, None, Op.add)
        ll_sb = pool.tile([1, 1], f32)
        nc.vector.scalar_tensor_tensor(ll_sb[:], ird[:], -UNIT_VOL,
                                       sl_sb[:], Op.mult, Op.add)
        nc.sync.dma_start(ll_o.unsqueeze(1), ll_sb[:])

    import concourse.hw_specs as hw_specs
    orig = bacc.get_activation_tables
    bacc.get_activation_tables = _patched_act_tables(hw_specs.get_activation_tables)
    try:
        nc.compile()
    finally:
        bacc.get_activation_tables = orig
    return nc


def _get_program(mu, alpha, beta, sigma, L):
    key = (float(mu), float(alpha), float(beta), float(sigma), L)
    if key not in _prog_cache:
        sig2 = float(sigma) * float(sigma)
        inv2sig2 = 1.0 / (2.0 * sig2)
        norm = float(alpha) * float(beta) / (2.0 * math.pi * sig2)
        nc = _build_program(float(mu), float(beta), inv2sig2, norm, L)
        consts = _const_arrays(L, norm, float(beta), inv2sig2)
        _prog_cache[key] = (nc, consts)
    return _prog_cache[key]


def kernel(x, mu, alpha, beta, sigma):
    from concourse.bass_utils import run_bass_kernel_spmd

    x = np.asarray(x, dtype=np.float32)
    B, L, _ = x.shape
    assert B == NCORES, f"expected batch {NCORES}, got {B}"

    nc, consts = _get_program(mu, alpha, beta, sigma, L)

    in_maps = []
    for b in range(B):
        m = dict(consts)
        m.update(_marshal_core_inputs(x[b, :, 0], x[b, :, 1], x[b, :, 2]))
        in_maps.append(m)

    res = run_bass_kernel_spmd(nc, in_maps, list(range(NCORES)))
    lams = np.stack([res.results[b]["lams_o"] for b in range(B)]).astype(np.float32)
    loglik = np.stack([res.results[b]["ll_o"][0] for b in range(B)]).astype(np.float32)
    return lams, loglik


# revision 24
# speedup vs baseline: 1.0199x; 1.0199x over previous
"""Spatio-temporal Hawkes process log-likelihood on Trainium2 (Bass/Tile).

Computes, for x[B, L, 3] = (t, s1, s2) and scalars mu/alpha/beta/sigma:
  lams[b, i]  = softplus(sum_{j<i} K(x_i, x_j) * 1[t_j>0] + mu) + 1e-5
  loglik[b]   = sum_i log(lams[b,i]) * 1[t_i>0]
              - UNIT_VOL * sum_{r, g} softplus(sum_j K((tt_r, ss_g), x_j) * m + mu)
with K(x, y) = norm * exp(-beta*(t_x - t_y) - |s_x - s_y|^2 / (2 sigma^2)),
norm = alpha*beta/(2 pi sigma^2), over a 50 x 50 x 50 (t, s1, s2) grid.

Strategy (one batch element per NeuronCore, 8 cores, data-parallel):
  The grid kernel factorizes in time AND is separable in space:
    K((tt_r, ss_g), x_j) = [norm * 1[0<t_j<=tt_r] * e^{beta(t_j-tt_r)}]
                         * e^{-inv2sig2 (g1_i-s1_j)^2} * e^{-inv2sig2 (g2_k-s2_j)^2}
  so per core we build two tiny per-axis tables A[j,i] (25 cols/half)
  and B[j,k] (50 cols) via K<=4 quadratic-expansion matmuls + ACT exp,
  expand G[j,g] = A*B with one broadcast-AP multiply, build the
  temporal weight matrix W[j,r], and get the softplus argument as
  W.T @ G on the PE (bf16 operands, fp32 PSUM accumulation).
  softplus+row-sum is chunk-pipelined against the matmuls, with the
  row sums riding the accum_out ports of the DVE relu and ACT log1p.
  The per-event [L, L] exponent is built by 2 accumulated K=2 fp32
  matmuls over pair-packed rank-1 factors (fp32 because the quadratic
  expansion cancels catastrophically in low precision), one ACT exp,
  and a masked row-reduce fused into a scalar_tensor_tensor.

  Partition packing: the 2500 spatial grid points are split in two
  halves of 1250; partitions 0:64 hold events-vs-half0, 64:128 hold
  events-vs-half1, so elementwise engines run at full 128-lane width.

softplus(v) is decomposed as v + log1p(exp(-v)) (exact for all v; the
linear part is summed analytically from Gsum, numerically safe since
v >= mu here), and the
activation-table map is patched during compile so every ACT func
(Exp/Ln/Abs/Copy) resolves to the single `natural_log_exp_and_others`
set -> one table load, no set thrashing.

All tiny per-core staging (dup columns, concatenated rows, packed
rank-1 factors) is marshalled host-side as pure copies - engines can
only address SBUF partition starts of 0/32/64/96, so single-row writes
at other partitions are not expressible on-device.  Emission order is
tuned so no in-order engine stream blocks on a late dependency.
"""

import math
import numpy as np
from contextlib import ExitStack

R = 50                      # INT_RES (time and each spatial axis)
RG = R * R                  # 2500 spatial grid points
HALF = RG // 2              # 1250
NCORES = 8
UNIT_VOL = 1.0 / float(R ** 3)
BIG_NEG = 1.0e30
CHUNKS = ((0, 512), (512, 512), (1024, HALF - 1024))

_prog_cache: dict = {}


def _const_arrays(L: int, norm: float, beta: float, inv2sig2: float):
    f32 = np.float32
    g1 = np.linspace(0.0, 1.0, R).astype(f32)
    g2 = np.linspace(0.0, 1.0, R).astype(f32)

    ccols = np.zeros((128, 2), f32)
    ccols[:, 0] = 1.0                                   # ones column
    ccols[:, 1] = (np.arange(128) % 64 < R)             # sel (valid r rows)

    ctril = (norm * np.tril(np.ones((L, L), np.float64), -1)).astype(f32)

    # crows [4, 256]:
    #   [0:1, 0:128]   ones row
    #   [0:1, 128:178] -linspace(0,1,R)
    #   [0:2, 178:180] per-event STT coefficient columns
    #   [0:6, 180:255] rhs of combined A|B table matmul (block diagonal):
    #       cols 0:25  rows [g1lo^2; g1hi^2; -2g1lo; -2g1hi; 0; 0]
    #       cols 25:75 rows [0; 0; 0; 0; g2^2; -2g2]
    crows = np.zeros((6, 256), f32)
    crows[0, 0:128] = 1.0
    crows[0, 128:178] = -np.linspace(0.0, 1.0, R)
    crows[0, 178] = -beta; crows[1, 178] = 2.0 * inv2sig2    # colA -> [u; a1]
    crows[0, 179] = beta;  crows[1, 179] = 2.0 * inv2sig2    # colB -> [v; a2]
    g1lo, g1hi = g1[0:25], g1[25:50]
    crows[0, 180:205] = g1lo ** 2
    crows[1, 180:205] = g1hi ** 2
    crows[2, 180:205] = -2.0 * g1lo
    crows[3, 180:205] = -2.0 * g1hi
    crows[4, 205:255] = g2 ** 2
    crows[5, 205:255] = -2.0 * g2
    return dict(ccols=ccols, ctril=ctril, crows=crows)


def _marshal_core_inputs(t, s1, s2):
    """Pure-layout staging of one sequence's inputs (no arithmetic).

    icols [128, 3]: t/s1/s2 duplicated into both partition halves.
    irows [8, 704]:
      [0:1, 0:192]   t | s1 | s2 concatenated rows
      [0:4, 192:320] lhsT of the A-table matmul
                     [ind_lo; ind_hi; s1*ind_lo; s1*ind_hi]
      [0:2, 320:384] [t; s1]   (pair-packed per-event row inputs)
      [0:2, 384:448] [t; s2]
      [0:2, 448:512] [ones; s1] (rhs of per-event matmul 1)
      [0:2, 512:576] [ones; s2] (lhsT of per-event matmul 2)
      [0:6, 192:320] K=6 lhsT of the combined A|B table matmul
                     [ind_lo; ind_hi; s1*ind_lo; s1*ind_hi; ones128; s2_dup]
    """
    f32 = np.float32
    L = t.shape[0]
    icols = np.zeros((128, 3), f32)
    icols[0:L, 0] = t; icols[64:64 + L, 0] = t
    icols[0:L, 1] = s1; icols[64:64 + L, 1] = s1
    icols[0:L, 2] = s2; icols[64:64 + L, 2] = s2
    irows = np.zeros((8, 704), f32)
    irows[0, 0:L] = t
    irows[0, L:2 * L] = s1
    irows[0, 2 * L:3 * L] = s2
    irows[0, 192:256] = 1.0                       # ind_lo
    irows[1, 256:320] = 1.0                       # ind_hi
    irows[2, 192:192 + L] = s1
    irows[3, 256:256 + L] = s1
    irows[0, 320:320 + L] = t;   irows[1, 320:320 + L] = s1
    irows[0, 384:384 + L] = t;   irows[1, 384:384 + L] = s2
    irows[0, 448:448 + L] = 1.0; irows[1, 448:448 + L] = s1
    irows[0, 512:512 + L] = 1.0; irows[1, 512:512 + L] = s2
    irows[4, 192:320] = 1.0
    irows[5, 192:192 + L] = s2;  irows[5, 256:256 + L] = s2
    return {"icols": icols, "irows": irows}


def _patched_act_tables(orig_fn, preferred="natural_log_exp_and_others"):
    """Wrap get_activation_tables so every function present in the
    preferred set resolves only to it (same names/order, so the emitted
    act_func_set_id still indexes the real act_info.json)."""
    import functools

    @functools.cache
    def wrapper(arch):
        tables = dict(orig_fn(arch))
        pref = tables.get(preferred)
        if not pref:
            return tables
        return {
            name: (funcs if name == preferred else funcs - pref)
            for name, funcs in tables.items()
        }
    return wrapper


def _build_program(mu: float, beta: float, inv2sig2: float, norm: float, L: int):
    import concourse.bass as bass
    import concourse.bacc as bacc
    import concourse.tile as tile
    import concourse.mybir as mybir

    f32 = mybir.dt.float32
    bf16 = mybir.dt.bfloat16
    Act = mybir.ActivationFunctionType
    Op = mybir.AluOpType

    nc = bacc.Bacc("TRN2", target_bir_lowering=False, debug=False,
                   enable_asserts=True, num_devices=NCORES)

    # ---- DRAM I/O
    icols_d = nc.dram_tensor("icols", [128, 3], f32, kind="ExternalInput").ap()
    irows_d = nc.dram_tensor("irows", [8, 704], f32, kind="ExternalInput").ap()
    ccols_d = nc.dram_tensor("ccols", [128, 2], f32, kind="ExternalInput").ap()
    ctril_d = nc.dram_tensor("ctril", [L, L], f32, kind="ExternalInput").ap()
    crows_d = nc.dram_tensor("crows", [6, 256], f32, kind="ExternalInput").ap()
    lams_o = nc.dram_tensor("lams_o", [L], f32, kind="ExternalOutput").ap()
    ll_o = nc.dram_tensor("ll_o", [1], f32, kind="ExternalOutput").ap()

    with tile.TileContext(nc) as tc, ExitStack() as ctx:
        pool = ctx.enter_context(tc.tile_pool(name="sbuf", bufs=1))
        cpool = ctx.enter_context(tc.tile_pool(name="chunk", bufs=2))
        psum = ctx.enter_context(tc.tile_pool(name="psum", bufs=1,
                                              space=bass.MemorySpace.PSUM))
        psmall = ctx.enter_context(tc.tile_pool(name="psmall", bufs=3,
                                                space=bass.MemorySpace.PSUM))

        # ---- loads (5 small DMAs on 3 queues)
        irows = pool.tile([8, 704], f32)
        nc.sync.dma_start(irows[:], irows_d[:])
        icols = pool.tile([128, 3], f32)
        nc.sync.dma_start(icols[:], icols_d[:])
        crows = pool.tile([6, 256], f32)
        nc.gpsimd.dma_start(crows[:], crows_d[:])
        ctril = pool.tile([L, L], f32)
        nc.gpsimd.dma_start(ctril[:], ctril_d[:])
        ccols = pool.tile([128, 2], f32)
        nc.scalar.dma_start(ccols[:], ccols_d[:])

        t_col = icols[:, 0:1]
        s1_col = icols[:, 1:2]
        s2_col = icols[:, 2:3]
        t_row = irows[0:1, 0:L]
        s1_row = irows[0:1, L:2 * L]
        s2_row = irows[0:1, 2 * L:3 * L]
        lhsT6AB = irows[0:6, 192:320]
        pA_in = irows[0:2, 320:320 + L]
        pB_in = irows[0:2, 384:384 + L]
        rhs_mm1 = irows[0:2, 448:448 + L]
        lhsT_mm2 = irows[0:2, 512:512 + L]

        ones_col = ccols[:, 0:1]
        sel_col = ccols[:, 1:2]
        ones_r = crows[0:1, 0:128]
        negttg = crows[0:1, 128:178]
        scoefA = crows[0:2, 178:179]
        scoefB = crows[0:2, 179:180]
        rhsAB = crows[0:6, 180:255]

        negmu_col = pool.tile([128, 1], f32)
        nc.vector.memset(negmu_col[:], -mu)

        # ---- per-partition spatial biases: -inv2sig2*s1^2, -inv2sig2*s2^2
        biasA = pool.tile([128, 1], f32)
        nc.vector.tensor_scalar(biasA[:], s1_col, s1_col, -inv2sig2,
                                Op.mult, Op.mult)
        biasB = pool.tile([128, 1], f32)
        nc.vector.tensor_scalar(biasB[:], s2_col, s2_col, -inv2sig2,
                                Op.mult, Op.mult)

        # ---- temporal weights W_T[j(packed), r] (norm folded into mask)
        bc_ps = psmall.tile([128, R], f32, tag="small")
        nc.tensor.matmul(bc_ps[:], ones_r, negttg, start=True, stop=True)
        dtW = pool.tile([128, R], f32)
        nc.vector.tensor_scalar(dtW[:], bc_ps[:], t_col, None, Op.add)
        Ew = pool.tile([128, R], f32)
        nc.scalar.activation(Ew[:], dtW[:], Act.Exp, scale=beta)
        hn_col = pool.tile([128, 1], f32)
        nc.vector.tensor_scalar(hn_col[:], t_col, 0.0, norm, Op.is_gt, Op.mult)
        h_col = pool.tile([128, 1], f32)
        nc.vector.tensor_scalar(h_col[:], t_col, 0.0, None, Op.is_gt)
        Mw = pool.tile([128, R], f32)
        nc.vector.tensor_scalar(Mw[:], dtW[:], 0.0, hn_col[:, 0:1], Op.is_le, Op.mult)
        WT2 = pool.tile([128, 64], bf16)
        nc.vector.memset(WT2[:], 0.0)
        nc.vector.tensor_tensor(WT2[:, 0:R], Ew[:], Mw[:], Op.mult)

        # ---- separable spatial kernel: G[p, i*50+k] = A[p, i] * B[p, k]
        psAB = psmall.tile([128, 80], f32, tag="small")
        nc.tensor.matmul(psAB[:, 0:75], lhsT6AB, rhsAB, start=True, stop=True)
        A2 = pool.tile([128, 25], f32)
        nc.scalar.activation(A2[:], psAB[:, 0:25], Act.Exp,
                             scale=-inv2sig2, bias=biasA[:, 0:1])
        B2 = pool.tile([128, R], f32)
        nc.scalar.activation(B2[:], psAB[:, 25:75], Act.Exp,
                             scale=-inv2sig2, bias=biasB[:, 0:1])
        G = pool.tile([128, HALF], bf16)

        def _emit_g(goff, gw):
            a0 = goff // R
            nc.vector.tensor_tensor(
                G[:, goff:goff + gw].rearrange("p (a b) -> p a b", a=gw // R),
                A2[:, a0:a0 + gw // R].unsqueeze(2).broadcast_to(
                    [128, gw // R, R]),
                B2[:].unsqueeze(1).broadcast_to([128, gw // R, R]),
                Op.mult)

        _emit_g(0, 600)

        # ---- per-event exponent via 2 accumulated K=2 matmuls (fp32)
        sq1 = pool.tile([1, L], f32)
        nc.vector.tensor_tensor(sq1[:], s1_row, s1_row, Op.mult)
        sq2 = pool.tile([1, L], f32)
        nc.vector.tensor_tensor(sq2[:], s2_row, s2_row, Op.mult)
        ssum = pool.tile([1, L], f32)
        nc.vector.tensor_tensor(ssum[:], sq1[:], sq2[:], Op.add)
        w2 = pool.tile([2, L], f32)
        nc.vector.memset(w2[:], 0.0)
        nc.vector.tensor_scalar(w2[0:1, :], ssum[:], -inv2sig2, None, Op.mult)

        pairA = pool.tile([2, L], f32)
        nc.vector.scalar_tensor_tensor(pairA[:], pA_in, scoefA, w2[:],
                                       Op.mult, Op.add)
        pairB = pool.tile([2, L], f32)
        nc.vector.scalar_tensor_tensor(pairB[:], pB_in, scoefB, w2[:],
                                       Op.mult, Op.add)
        ha = pool.tile([1, L], f32)
        nc.vector.tensor_scalar(ha[:], t_row, 0.0, 1.0, Op.is_gt, Op.subtract)
        hm = pool.tile([1, L], f32)
        nc.vector.tensor_scalar(hm[:], ha[:], BIG_NEG, None, Op.mult)
        nc.vector.tensor_tensor(pairB[0:1, :], pairB[0:1, :], hm[:], Op.add)

        zev_ps = psmall.tile([L, L], f32, tag="small")
        nc.tensor.matmul(zev_ps[:], pairA[:], rhs_mm1, start=True, stop=False)
        nc.tensor.matmul(zev_ps[:], lhsT_mm2, pairB[:], start=False, stop=True)

        Ke = pool.tile([L, L], f32)
        nc.scalar.activation(Ke[:], zev_ps[:], Act.Exp)
        Km = pool.tile([L, L], f32)
        lam_col = pool.tile([L, 1], f32)
        nc.vector.scalar_tensor_tensor(Km[:], Ke[:], 0.0, ctril[:],
                                       Op.add, Op.mult, accum_out=lam_col[:])
        _emit_g(600, 650)
        # row sums of G (for the analytic relu part of softplus):
        # Gsum[p] = sum_i A2[p,i] * sum_k B2[p,k]
        sA = pool.tile([128, 1], f32)
        nc.vector.tensor_reduce(sA[:], A2[:], mybir.AxisListType.X, Op.add)
        sB = pool.tile([128, 1], f32)
        nc.vector.tensor_reduce(sB[:], B2[:], mybir.AxisListType.X, Op.add)
        gsum = pool.tile([128, 1], bf16)
        nc.vector.tensor_tensor(gsum[:], sA[:], sB[:], Op.mult)

        # lams = softplus(lam_raw + mu) + 1e-5; lam_raw >= 0 and mu >= 0, so
        # softplus(v) = v + log1p(exp(-v)) with no abs/relu needed
        ee = pool.tile([L, 1], f32)
        nc.scalar.activation(ee[:], lam_col[:], Act.Exp, scale=-1.0,
                             bias=negmu_col[0:L, 0:1])
        lp = pool.tile([L, 1], f32)
        nc.scalar.activation(lp[:], ee[:], Act.Ln, bias=1.0)
        lams_col = pool.tile([L, 1], f32)
        nc.vector.scalar_tensor_tensor(lams_col[:], lam_col[:], mu + 1e-5,
                                       lp[:], Op.add, Op.add)
        nc.sync.dma_start(lams_o.unsqueeze(1), lams_col[:])
        lnl = pool.tile([L, 1], f32)
        nc.scalar.activation(lnl[:], lams_col[:], Act.Ln)
        lnm = pool.tile([L, 1], f32)
        nc.vector.tensor_tensor(lnm[:], lnl[:], h_col[0:L, 0:1], Op.mult)

        # ---- grid: matmuls per 512 PSUM bank; softplus pipelined in two
        #      625-wide chunks (bank-level deps let them overlap the mms).
        # v = z + mu >= 0 always (z is a sum of nonnegative products and
        # mu >= 0), so softplus(v) = v + log1p(exp(-v)); the linear part
        # is summed analytically via Gsum below, only log1p runs per-element.
        # acc cols 0:3 = per-chunk log1p row sums
        acc = pool.tile([128, 3], f32)
        zz = pool.tile([128, 512], f32)
        nc.vector.memset(zz[:], 0.0)
        z_ps = psum.tile([128, 1536], f32, tag="z")
        SP_CHUNKS = CHUNKS
        emitted = set()

        def _mms_for(lo, hi):
            for moff, mw in CHUNKS:
                if moff in emitted or moff >= hi or moff + mw <= lo:
                    continue
                emitted.add(moff)
                for h in (0, 1):
                    p0 = h * 64
                    nc.tensor.matmul(z_ps[p0:p0 + 64, moff:moff + mw],
                                     WT2[p0:p0 + 64, 0:64],
                                     G[p0:p0 + 64, moff:moff + mw],
                                     start=True, stop=True)

        for ci, (off, w) in enumerate(SP_CHUNKS):
            _mms_for(off, off + w)
            e_c = cpool.tile([128, 640], f32, tag="e_c")
            nc.scalar.activation(e_c[:, 0:w], z_ps[:, off:off + w], Act.Exp,
                                 scale=-1.0, bias=negmu_col[:, 0:1])
            l_c = cpool.tile([128, 640], f32, tag="l_c")
            nc.scalar.activation(l_c[:, 0:w], e_c[:, 0:w], Act.Ln, bias=1.0)
            s_c = cpool.tile([128, 640], f32, tag="s_c")
            nc.vector.scalar_tensor_tensor(s_c[:, 0:w], l_c[:, 0:w], 0.0,
                                           zz[:, 0:w], Op.add, Op.add,
                                           accum_out=acc[:, ci:ci + 1])

        # ---- reductions and final combine
        # zsum[r] = sum_g z[r, g] via Gsum (both packed halves accumulate)
        zs_ps = psmall.tile([64, 1], f32, tag="small")
        nc.tensor.matmul(zs_ps[:], WT2[0:64, 0:64], gsum[0:64, 0:1],
                         start=True, stop=False)
        nc.tensor.matmul(zs_ps[:], WT2[64:128, 0:64], gsum[64:128, 0:1],
                         start=False, stop=True)
        zc = pool.tile([64, 1], f32)
        nc.vector.tensor_copy(zc[:], zs_ps[:])

        red_ps = psmall.tile([1, 8], f32, tag="small")
        nc.tensor.matmul(red_ps[0:1, 0:1], lnm[:, 0:1], ones_col[0:L, 0:1],
                         start=True, stop=True)
        nc.tensor.matmul(red_ps[0:1, 1:2], zc[:, 0:1], ones_col[0:L, 0:1],
                         start=True, stop=True)
        nc.tensor.matmul(red_ps[0:1, 2:5], sel_col, acc[:, 0:3],
                         start=True, stop=True)

        # loglik = sumlog - V*(zsum_tot + 125000*mu + l1p_tot)
        ird = pool.tile([1, 1], f32)
        nc.vector.tensor_reduce(ird[:], red_ps[0:1, 1:5],
                                mybir.AxisListType.X, Op.add)
        sl_sb = pool.tile([1, 1], f32)
        nc.vector.tensor_scalar(sl_sb[:], red_ps[0:1, 0:1],
                                -UNIT_VOL * float(RG * R) * mu, None, Op.add)
        ll_sb = pool.tile([1, 1], f32)
        nc.vector.scalar_tensor_tensor(ll_sb[:], ird[:], -UNIT_VOL,
                                       sl_sb[:], Op.mult, Op.add)
        nc.sync.dma_start(ll_o.unsqueeze(1), ll_sb[:])

    import concourse.hw_specs as hw_specs
    orig = bacc.get_activation_tables
    bacc.get_activation_tables = _patched_act_tables(hw_specs.get_activation_tables)
    try:
        nc.compile()
    finally:
        bacc.get_activation_tables = orig
    return nc


def _get_program(mu, alpha, beta, sigma, L):
    key = (float(mu), float(alpha), float(beta), float(sigma), L)
    if key not in _prog_cache:
        sig2 = float(sigma) * float(sigma)
        inv2sig2 = 1.0 / (2.0 * sig2)
        norm = float(alpha) * float(beta) / (2.0 * math.pi * sig2)
        nc = _build_program(float(mu), float(beta), inv2sig2, norm, L)
        consts = _const_arrays(L, norm, float(beta), inv2sig2)
        _prog_cache[key] = (nc, consts)
    return _prog_cache[key]


def kernel(x, mu, alpha, beta, sigma):
    from concourse.bass_utils import run_bass_kernel_spmd

    x = np.asarray(x, dtype=np.float32)
    B, L, _ = x.shape
    assert B == NCORES, f"expected batch {NCORES}, got {B}"

    nc, consts = _get_program(mu, alpha, beta, sigma, L)

    in_maps = []
    for b in range(B):
        m = dict(consts)
        m.update(_marshal_core_inputs(x[b, :, 0], x[b, :, 1], x[b, :, 2]))
        in_maps.append(m)

    res = run_bass_kernel_spmd(nc, in_maps, list(range(NCORES)))
    lams = np.stack([res.results[b]["lams_o"] for b in range(B)]).astype(np.float32)
    loglik = np.stack([res.results[b]["ll_o"][0] for b in range(B)]).astype(np.float32)
    return lams, loglik


# revision 25
# speedup vs baseline: 1.0243x; 1.0044x over previous
"""Spatio-temporal Hawkes process log-likelihood on Trainium2 (Bass/Tile).

Computes, for x[B, L, 3] = (t, s1, s2) and scalars mu/alpha/beta/sigma:
  lams[b, i]  = softplus(sum_{j<i} K(x_i, x_j) * 1[t_j>0] + mu) + 1e-5
  loglik[b]   = sum_i log(lams[b,i]) * 1[t_i>0]
              - UNIT_VOL * sum_{r, g} softplus(sum_j K((tt_r, ss_g), x_j) * m + mu)
with K(x, y) = norm * exp(-beta*(t_x - t_y) - |s_x - s_y|^2 / (2 sigma^2)),
norm = alpha*beta/(2 pi sigma^2), over a 50 x 50 x 50 (t, s1, s2) grid.

Strategy (one batch element per NeuronCore, 8 cores, data-parallel):
  The grid kernel factorizes in time AND is separable in space:
    K((tt_r, ss_g), x_j) = [norm * 1[0<t_j<=tt_r] * e^{beta(t_j-tt_r)}]
                         * e^{-inv2sig2 (g1_i-s1_j)^2} * e^{-inv2sig2 (g2_k-s2_j)^2}
  so per core we build two tiny per-axis tables A[j,i] (25 cols/half)
  and B[j,k] (50 cols) via K<=4 quadratic-expansion matmuls + ACT exp,
  expand G[j,g] = A*B with one broadcast-AP multiply, build the
  temporal weight matrix W[j,r], and get the softplus argument as
  W.T @ G on the PE (bf16 operands, fp32 PSUM accumulation).
  softplus+row-sum is chunk-pipelined against the matmuls, with the
  row sums riding the accum_out ports of the DVE relu and ACT log1p.
  The per-event [L, L] exponent is built by 2 accumulated K=2 fp32
  matmuls over pair-packed rank-1 factors (fp32 because the quadratic
  expansion cancels catastrophically in low precision), one ACT exp,
  and a masked row-reduce fused into a scalar_tensor_tensor.

  Partition packing: the 2500 spatial grid points are split in two
  halves of 1250; partitions 0:64 hold events-vs-half0, 64:128 hold
  events-vs-half1, so elementwise engines run at full 128-lane width.

softplus(v) is decomposed as v + log1p(exp(-v)) (exact for all v; the
linear part is summed analytically from Gsum, numerically safe since
v >= mu here), and the
activation-table map is patched during compile so every ACT func
(Exp/Ln/Abs/Copy) resolves to the single `natural_log_exp_and_others`
set -> one table load, no set thrashing.

All tiny per-core staging (dup columns, concatenated rows, packed
rank-1 factors) is marshalled host-side as pure copies - engines can
only address SBUF partition starts of 0/32/64/96, so single-row writes
at other partitions are not expressible on-device.  Emission order is
tuned so no in-order engine stream blocks on a late dependency.
"""

import math
import numpy as np
from contextlib import ExitStack

R = 50                      # INT_RES (time and each spatial axis)
RG = R * R                  # 2500 spatial grid points
HALF = RG // 2              # 1250
NCORES = 8
UNIT_VOL = 1.0 / float(R ** 3)
BIG_NEG = 1.0e30
CHUNKS = ((0, 512), (512, 512), (1024, HALF - 1024))

_prog_cache: dict = {}


def _const_arrays(L: int, norm: float, beta: float, inv2sig2: float):
    f32 = np.float32
    g1 = np.linspace(0.0, 1.0, R).astype(f32)
    g2 = np.linspace(0.0, 1.0, R).astype(f32)

    ccols = np.zeros((128, 2), f32)
    ccols[:, 0] = 1.0                                   # ones column
    ccols[:, 1] = (np.arange(128) % 64 < R)             # sel (valid r rows)

    ctril = (norm * np.tril(np.ones((L, L), np.float64), -1)).astype(f32)

    # crows [4, 256]:
    #   [0:1, 0:128]   ones row
    #   [0:1, 128:178] -linspace(0,1,R)
    #   [0:2, 178:180] per-event STT coefficient columns
    #   [0:6, 180:255] rhs of combined A|B table matmul (block diagonal):
    #       cols 0:25  rows [g1lo^2; g1hi^2; -2g1lo; -2g1hi; 0; 0]
    #       cols 25:75 rows [0; 0; 0; 0; g2^2; -2g2]
    crows = np.zeros((6, 256), f32)
    crows[0, 0:128] = 1.0
    crows[0, 128:178] = -np.linspace(0.0, 1.0, R)
    crows[0, 178] = -beta; crows[1, 178] = 2.0 * inv2sig2    # colA -> [u; a1]
    crows[0, 179] = beta;  crows[1, 179] = 2.0 * inv2sig2    # colB -> [v; a2]
    g1lo, g1hi = g1[0:25], g1[25:50]
    crows[0, 180:205] = g1lo ** 2
    crows[1, 180:205] = g1hi ** 2
    crows[2, 180:205] = -2.0 * g1lo
    crows[3, 180:205] = -2.0 * g1hi
    crows[4, 205:255] = g2 ** 2
    crows[5, 205:255] = -2.0 * g2
    return dict(ccols=ccols, ctril=ctril, crows=crows)


def _marshal_core_inputs(t, s1, s2):
    """Pure-layout staging of one sequence's inputs (no arithmetic).

    icols [128, 3]: t/s1/s2 duplicated into both partition halves.
    irows [8, 704]:
      [0:1, 0:192]   t | s1 | s2 concatenated rows
      [0:4, 192:320] lhsT of the A-table matmul
                     [ind_lo; ind_hi; s1*ind_lo; s1*ind_hi]
      [0:2, 320:384] [t; s1]   (pair-packed per-event row inputs)
      [0:2, 384:448] [t; s2]
      [0:2, 448:512] [ones; s1] (rhs of per-event matmul 1)
      [0:2, 512:576] [ones; s2] (lhsT of per-event matmul 2)
      [0:6, 192:320] K=6 lhsT of the combined A|B table matmul
                     [ind_lo; ind_hi; s1*ind_lo; s1*ind_hi; ones128; s2_dup]
    """
    f32 = np.float32
    L = t.shape[0]
    icols = np.zeros((128, 3), f32)
    icols[0:L, 0] = t; icols[64:64 + L, 0] = t
    icols[0:L, 1] = s1; icols[64:64 + L, 1] = s1
    icols[0:L, 2] = s2; icols[64:64 + L, 2] = s2
    irows = np.zeros((8, 704), f32)
    irows[0, 0:L] = t
    irows[0, L:2 * L] = s1
    irows[0, 2 * L:3 * L] = s2
    irows[0, 192:256] = 1.0                       # ind_lo
    irows[1, 256:320] = 1.0                       # ind_hi
    irows[2, 192:192 + L] = s1
    irows[3, 256:256 + L] = s1
    irows[0, 320:320 + L] = t;   irows[1, 320:320 + L] = s1
    irows[0, 384:384 + L] = t;   irows[1, 384:384 + L] = s2
    irows[0, 448:448 + L] = 1.0; irows[1, 448:448 + L] = s1
    irows[0, 512:512 + L] = 1.0; irows[1, 512:512 + L] = s2
    irows[4, 192:320] = 1.0
    irows[5, 192:192 + L] = s2;  irows[5, 256:256 + L] = s2
    return {"icols": icols, "irows": irows}


def _patched_act_tables(orig_fn, preferred="natural_log_exp_and_others"):
    """Wrap get_activation_tables so every function present in the
    preferred set resolves only to it (same names/order, so the emitted
    act_func_set_id still indexes the real act_info.json)."""
    import functools

    @functools.cache
    def wrapper(arch):
        tables = dict(orig_fn(arch))
        pref = tables.get(preferred)
        if not pref:
            return tables
        return {
            name: (funcs if name == preferred else funcs - pref)
            for name, funcs in tables.items()
        }
    return wrapper


def _build_program(mu: float, beta: float, inv2sig2: float, norm: float, L: int):
    import concourse.bass as bass
    import concourse.bacc as bacc
    import concourse.tile as tile
    import concourse.mybir as mybir

    f32 = mybir.dt.float32
    bf16 = mybir.dt.bfloat16
    Act = mybir.ActivationFunctionType
    Op = mybir.AluOpType

    nc = bacc.Bacc("TRN2", target_bir_lowering=False, debug=False,
                   enable_asserts=True, num_devices=NCORES)

    # ---- DRAM I/O
    icols_d = nc.dram_tensor("icols", [128, 3], f32, kind="ExternalInput").ap()
    irows_d = nc.dram_tensor("irows", [8, 704], f32, kind="ExternalInput").ap()
    ccols_d = nc.dram_tensor("ccols", [128, 2], f32, kind="ExternalInput").ap()
    ctril_d = nc.dram_tensor("ctril", [L, L], f32, kind="ExternalInput").ap()
    crows_d = nc.dram_tensor("crows", [6, 256], f32, kind="ExternalInput").ap()
    lams_o = nc.dram_tensor("lams_o", [L], f32, kind="ExternalOutput").ap()
    ll_o = nc.dram_tensor("ll_o", [1], f32, kind="ExternalOutput").ap()

    with tile.TileContext(nc) as tc, ExitStack() as ctx:
        pool = ctx.enter_context(tc.tile_pool(name="sbuf", bufs=1))
        cpool = ctx.enter_context(tc.tile_pool(name="chunk", bufs=3))
        psum = ctx.enter_context(tc.tile_pool(name="psum", bufs=1,
                                              space=bass.MemorySpace.PSUM))
        psmall = ctx.enter_context(tc.tile_pool(name="psmall", bufs=3,
                                                space=bass.MemorySpace.PSUM))

        # ---- loads (5 small DMAs on 3 queues)
        irows = pool.tile([8, 704], f32)
        nc.sync.dma_start(irows[:], irows_d[:])
        icols = pool.tile([128, 3], f32)
        nc.sync.dma_start(icols[:], icols_d[:])
        crows = pool.tile([6, 256], f32)
        nc.gpsimd.dma_start(crows[:], crows_d[:])
        ctril = pool.tile([L, L], f32)
        nc.gpsimd.dma_start(ctril[:], ctril_d[:])
        ccols = pool.tile([128, 2], f32)
        nc.scalar.dma_start(ccols[:], ccols_d[:])

        t_col = icols[:, 0:1]
        s1_col = icols[:, 1:2]
        s2_col = icols[:, 2:3]
        t_row = irows[0:1, 0:L]
        s1_row = irows[0:1, L:2 * L]
        s2_row = irows[0:1, 2 * L:3 * L]
        lhsT6AB = irows[0:6, 192:320]
        pA_in = irows[0:2, 320:320 + L]
        pB_in = irows[0:2, 384:384 + L]
        rhs_mm1 = irows[0:2, 448:448 + L]
        lhsT_mm2 = irows[0:2, 512:512 + L]

        ones_col = ccols[:, 0:1]
        sel_col = ccols[:, 1:2]
        ones_r = crows[0:1, 0:128]
        negttg = crows[0:1, 128:178]
        scoefA = crows[0:2, 178:179]
        scoefB = crows[0:2, 179:180]
        rhsAB = crows[0:6, 180:255]

        negmu_col = pool.tile([128, 1], f32)
        nc.vector.memset(negmu_col[:], -mu)

        # ---- per-partition spatial biases: -inv2sig2*s1^2, -inv2sig2*s2^2
        biasA = pool.tile([128, 1], f32)
        nc.vector.tensor_scalar(biasA[:], s1_col, s1_col, -inv2sig2,
                                Op.mult, Op.mult)
        biasB = pool.tile([128, 1], f32)
        nc.vector.tensor_scalar(biasB[:], s2_col, s2_col, -inv2sig2,
                                Op.mult, Op.mult)

        # ---- temporal weights W_T[j(packed), r] (norm folded into mask)
        bc_ps = psmall.tile([128, R], f32, tag="small")
        nc.tensor.matmul(bc_ps[:], ones_r, negttg, start=True, stop=True)
        dtW = pool.tile([128, R], f32)
        nc.vector.tensor_scalar(dtW[:], bc_ps[:], t_col, None, Op.add)
        Ew = pool.tile([128, R], f32)
        nc.scalar.activation(Ew[:], dtW[:], Act.Exp, scale=beta)
        hn_col = pool.tile([128, 1], f32)
        nc.vector.tensor_scalar(hn_col[:], t_col, 0.0, norm, Op.is_gt, Op.mult)
        h_col = pool.tile([128, 1], f32)
        nc.vector.tensor_scalar(h_col[:], t_col, 0.0, None, Op.is_gt)
        Mw = pool.tile([128, R], f32)
        nc.vector.tensor_scalar(Mw[:], dtW[:], 0.0, hn_col[:, 0:1], Op.is_le, Op.mult)
        WT2 = pool.tile([128, 64], bf16)
        nc.vector.memset(WT2[:], 0.0)
        nc.vector.tensor_tensor(WT2[:, 0:R], Ew[:], Mw[:], Op.mult)

        # ---- separable spatial kernel: G[p, i*50+k] = A[p, i] * B[p, k]
        psAB = psmall.tile([128, 80], f32, tag="small")
        nc.tensor.matmul(psAB[:, 0:75], lhsT6AB, rhsAB, start=True, stop=True)
        A2 = pool.tile([128, 25], f32)
        nc.scalar.activation(A2[:], psAB[:, 0:25], Act.Exp,
                             scale=-inv2sig2, bias=biasA[:, 0:1])
        B2 = pool.tile([128, R], f32)
        nc.scalar.activation(B2[:], psAB[:, 25:75], Act.Exp,
                             scale=-inv2sig2, bias=biasB[:, 0:1])
        G = pool.tile([128, HALF], bf16)

        def _emit_g(goff, gw):
            a0 = goff // R
            nc.vector.tensor_tensor(
                G[:, goff:goff + gw].rearrange("p (a b) -> p a b", a=gw // R),
                A2[:, a0:a0 + gw // R].unsqueeze(2).broadcast_to(
                    [128, gw // R, R]),
                B2[:].unsqueeze(1).broadcast_to([128, gw // R, R]),
                Op.mult)

        _emit_g(0, 600)

        # ---- per-event exponent via 2 accumulated K=2 matmuls (fp32)
        sq1 = pool.tile([1, L], f32)
        nc.vector.tensor_tensor(sq1[:], s1_row, s1_row, Op.mult)
        sq2 = pool.tile([1, L], f32)
        nc.vector.tensor_tensor(sq2[:], s2_row, s2_row, Op.mult)
        ssum = pool.tile([1, L], f32)
        nc.vector.tensor_tensor(ssum[:], sq1[:], sq2[:], Op.add)
        w2 = pool.tile([2, L], f32)
        nc.vector.memset(w2[:], 0.0)
        nc.vector.tensor_scalar(w2[0:1, :], ssum[:], -inv2sig2, None, Op.mult)

        pairA = pool.tile([2, L], f32)
        nc.vector.scalar_tensor_tensor(pairA[:], pA_in, scoefA, w2[:],
                                       Op.mult, Op.add)
        pairB = pool.tile([2, L], f32)
        nc.vector.scalar_tensor_tensor(pairB[:], pB_in, scoefB, w2[:],
                                       Op.mult, Op.add)
        ha = pool.tile([1, L], f32)
        nc.vector.tensor_scalar(ha[:], t_row, 0.0, 1.0, Op.is_gt, Op.subtract)
        hm = pool.tile([1, L], f32)
        nc.vector.tensor_scalar(hm[:], ha[:], BIG_NEG, None, Op.mult)
        nc.vector.tensor_tensor(pairB[0:1, :], pairB[0:1, :], hm[:], Op.add)

        zev_ps = psmall.tile([L, L], f32, tag="small")
        nc.tensor.matmul(zev_ps[:], pairA[:], rhs_mm1, start=True, stop=False)
        nc.tensor.matmul(zev_ps[:], lhsT_mm2, pairB[:], start=False, stop=True)

        Ke = pool.tile([L, L], f32)
        nc.scalar.activation(Ke[:], zev_ps[:], Act.Exp)
        Km = pool.tile([L, L], f32)
        lam_col = pool.tile([L, 1], f32)
        nc.vector.scalar_tensor_tensor(Km[:], Ke[:], 0.0, ctril[:],
                                       Op.add, Op.mult, accum_out=lam_col[:])
        _emit_g(600, 650)
        # row sums of G (for the analytic relu part of softplus):
        # Gsum[p] = sum_i A2[p,i] * sum_k B2[p,k]
        sA = pool.tile([128, 1], f32)
        nc.vector.tensor_reduce(sA[:], A2[:], mybir.AxisListType.X, Op.add)
        sB = pool.tile([128, 1], f32)
        nc.vector.tensor_reduce(sB[:], B2[:], mybir.AxisListType.X, Op.add)
        gsum = pool.tile([128, 1], bf16)
        nc.vector.tensor_tensor(gsum[:], sA[:], sB[:], Op.mult)

        # lams = softplus(lam_raw + mu) + 1e-5; lam_raw >= 0 and mu >= 0, so
        # softplus(v) = v + log1p(exp(-v)) with no abs/relu needed
        ee = pool.tile([L, 1], f32)
        nc.scalar.activation(ee[:], lam_col[:], Act.Exp, scale=-1.0,
                             bias=negmu_col[0:L, 0:1])
        lp = pool.tile([L, 1], f32)
        nc.scalar.activation(lp[:], ee[:], Act.Ln, bias=1.0)
        lams_col = pool.tile([L, 1], f32)
        nc.vector.scalar_tensor_tensor(lams_col[:], lam_col[:], mu + 1e-5,
                                       lp[:], Op.add, Op.add)
        nc.sync.dma_start(lams_o.unsqueeze(1), lams_col[:])
        lnl = pool.tile([L, 1], f32)
        nc.scalar.activation(lnl[:], lams_col[:], Act.Ln)
        lnm = pool.tile([L, 1], f32)
        nc.vector.tensor_tensor(lnm[:], lnl[:], h_col[0:L, 0:1], Op.mult)

        # ---- grid: matmuls per 512 PSUM bank; softplus pipelined in two
        #      625-wide chunks (bank-level deps let them overlap the mms).
        # v = z + mu >= 0 always (z is a sum of nonnegative products and
        # mu >= 0), so softplus(v) = v + log1p(exp(-v)); the linear part
        # is summed analytically via Gsum below, only log1p runs per-element.
        # acc cols 0:3 = per-chunk log1p row sums
        acc = pool.tile([128, 3], f32)
        zz = pool.tile([128, 512], f32)
        nc.vector.memset(zz[:], 0.0)
        z_ps = psum.tile([128, 1536], f32, tag="z")
        SP_CHUNKS = CHUNKS
        emitted = set()

        def _mms_for(lo, hi):
            for moff, mw in CHUNKS:
                if moff in emitted or moff >= hi or moff + mw <= lo:
                    continue
                emitted.add(moff)
                for h in (0, 1):
                    p0 = h * 64
                    nc.tensor.matmul(z_ps[p0:p0 + 64, moff:moff + mw],
                                     WT2[p0:p0 + 64, 0:64],
                                     G[p0:p0 + 64, moff:moff + mw],
                                     start=True, stop=True)

        for ci, (off, w) in enumerate(SP_CHUNKS):
            _mms_for(off, off + w)
            e_c = cpool.tile([128, 640], f32, tag="e_c")
            nc.scalar.activation(e_c[:, 0:w], z_ps[:, off:off + w], Act.Exp,
                                 scale=-1.0, bias=negmu_col[:, 0:1])
            l_c = cpool.tile([128, 640], f32, tag="l_c")
            nc.scalar.activation(l_c[:, 0:w], e_c[:, 0:w], Act.Ln, bias=1.0)
            s_c = cpool.tile([128, 640], f32, tag="s_c")
            nc.vector.scalar_tensor_tensor(s_c[:, 0:w], l_c[:, 0:w], 0.0,
                                           zz[:, 0:w], Op.add, Op.add,
                                           accum_out=acc[:, ci:ci + 1])

        # ---- reductions and final combine
        # zsum[r] = sum_g z[r, g] via Gsum (both packed halves accumulate)
        zs_ps = psmall.tile([64, 1], f32, tag="small")
        nc.tensor.matmul(zs_ps[:], WT2[0:64, 0:64], gsum[0:64, 0:1],
                         start=True, stop=False)
        nc.tensor.matmul(zs_ps[:], WT2[64:128, 0:64], gsum[64:128, 0:1],
                         start=False, stop=True)
        zc = pool.tile([64, 1], f32)
        nc.vector.tensor_copy(zc[:], zs_ps[:])

        red_ps = psmall.tile([1, 8], f32, tag="small")
        nc.tensor.matmul(red_ps[0:1, 0:1], lnm[:, 0:1], ones_col[0:L, 0:1],
                         start=True, stop=True)
        sl_sb = pool.tile([1, 1], f32)
        nc.vector.tensor_scalar(sl_sb[:], red_ps[0:1, 0:1],
                                -UNIT_VOL * float(RG * R) * mu, None, Op.add)
        nc.tensor.matmul(red_ps[0:1, 1:2], zc[:, 0:1], ones_col[0:L, 0:1],
                         start=True, stop=True)
        nc.tensor.matmul(red_ps[0:1, 2:5], sel_col, acc[:, 0:3],
                         start=True, stop=True)

        # loglik = sumlog - V*(zsum_tot + 125000*mu + l1p_tot)
        ird = pool.tile([1, 1], f32)
        nc.vector.tensor_reduce(ird[:], red_ps[0:1, 1:5],
                                mybir.AxisListType.X, Op.add)
        ll_sb = pool.tile([1, 1], f32)
        nc.vector.scalar_tensor_tensor(ll_sb[:], ird[:], -UNIT_VOL,
                                       sl_sb[:], Op.mult, Op.add)
        nc.sync.dma_start(ll_o.unsqueeze(1), ll_sb[:])

    import concourse.hw_specs as hw_specs
    orig = bacc.get_activation_tables
    bacc.get_activation_tables = _patched_act_tables(hw_specs.get_activation_tables)
    try:
        nc.compile()
    finally:
        bacc.get_activation_tables = orig
    return nc


def _get_program(mu, alpha, beta, sigma, L):
    key = (float(mu), float(alpha), float(beta), float(sigma), L)
    if key not in _prog_cache:
        sig2 = float(sigma) * float(sigma)
        inv2sig2 = 1.0 / (2.0 * sig2)
        norm = float(alpha) * float(beta) / (2.0 * math.pi * sig2)
        nc = _build_program(float(mu), float(beta), inv2sig2, norm, L)
        consts = _const_arrays(L, norm, float(beta), inv2sig2)
        _prog_cache[key] = (nc, consts)
    return _prog_cache[key]


def kernel(x, mu, alpha, beta, sigma):
    from concourse.bass_utils import run_bass_kernel_spmd

    x = np.asarray(x, dtype=np.float32)
    B, L, _ = x.shape
    assert B == NCORES, f"expected batch {NCORES}, got {B}"

    nc, consts = _get_program(mu, alpha, beta, sigma, L)

    in_maps = []
    for b in range(B):
        m = dict(consts)
        m.update(_marshal_core_inputs(x[b, :, 0], x[b, :, 1], x[b, :, 2]))
        in_maps.append(m)

    res = run_bass_kernel_spmd(nc, in_maps, list(range(NCORES)))
    lams = np.stack([res.results[b]["lams_o"] for b in range(B)]).astype(np.float32)
    loglik = np.stack([res.results[b]["ll_o"][0] for b in range(B)]).astype(np.float32)
    return lams, loglik


# revision 32
# speedup vs baseline: 1.0644x; 1.0392x over previous
"""Spatio-temporal Hawkes process log-likelihood on Trainium2 (Bass/Tile).

Computes, for x[B, L, 3] = (t, s1, s2) and scalars mu/alpha/beta/sigma:
  lams[b, i]  = softplus(sum_{j<i} K(x_i, x_j) * 1[t_j>0] + mu) + 1e-5
  loglik[b]   = sum_i log(lams[b,i]) * 1[t_i>0]
              - UNIT_VOL * sum_{r, g} softplus(sum_j K((tt_r, ss_g), x_j) * m + mu)
with K(x, y) = norm * exp(-beta*(t_x - t_y) - |s_x - s_y|^2 / (2 sigma^2)),
norm = alpha*beta/(2 pi sigma^2), over a 50 x 50 x 50 (t, s1, s2) grid.

Strategy (one batch element per NeuronCore, 8 cores, data-parallel):
  The grid kernel factorizes in time AND is separable in space:
    K((tt_r, ss_g), x_j) = [norm * 1[0<t_j<=tt_r] * e^{beta(t_j-tt_r)}]
                         * e^{-inv2sig2 (g1_i-s1_j)^2} * e^{-inv2sig2 (g2_k-s2_j)^2}
  so per core we build two tiny per-axis tables A[j,i] (25 cols/half)
  and B[j,k] (50 cols) via K<=4 quadratic-expansion matmuls + ACT exp,
  expand G[j,g] = A*B with one broadcast-AP multiply, build the
  temporal weight matrix W[j,r], and get the softplus argument as
  W.T @ G on the PE (bf16 operands, fp32 PSUM accumulation).
  softplus+row-sum is chunk-pipelined against the matmuls, with the
  row sums riding the accum_out ports of the DVE relu and ACT log1p.
  The per-event [L, L] exponent is built by 2 accumulated K=2 fp32
  matmuls over pair-packed rank-1 factors (fp32 because the quadratic
  expansion cancels catastrophically in low precision), one ACT exp,
  and a masked row-reduce fused into a scalar_tensor_tensor.

  Partition packing: the 2500 spatial grid points are split in two
  halves of 1250; partitions 0:64 hold events-vs-half0, 64:128 hold
  events-vs-half1, so elementwise engines run at full 128-lane width.

softplus(v) is decomposed as v + log1p(exp(-v)) (exact for all v; the
linear part is summed analytically from Gsum, numerically safe since
v >= mu here), and the
activation-table map is patched during compile so every ACT func
(Exp/Ln/Abs/Copy) resolves to the single `natural_log_exp_and_others`
set -> one table load, no set thrashing.

All tiny per-core staging (dup columns, concatenated rows, packed
rank-1 factors) is marshalled host-side as pure copies - engines can
only address SBUF partition starts of 0/32/64/96, so single-row writes
at other partitions are not expressible on-device.  Emission order is
tuned so no in-order engine stream blocks on a late dependency.
"""

import math
import numpy as np
from contextlib import ExitStack

R = 50                      # INT_RES (time and each spatial axis)
RG = R * R                  # 2500 spatial grid points
HALF = RG // 2              # 1250
NCORES = 8
UNIT_VOL = 1.0 / float(R ** 3)
BIG_NEG = 1.0e30
CHUNKS = ((0, 512), (512, 512), (1024, HALF - 1024))

_prog_cache: dict = {}


def _const_arrays(L: int, norm: float, beta: float, inv2sig2: float):
    f32 = np.float32
    g1 = np.linspace(0.0, 1.0, R).astype(f32)
    g2 = np.linspace(0.0, 1.0, R).astype(f32)

    ccols = np.zeros((128, 2), f32)
    ccols[:, 0] = 1.0                                   # ones column
    ccols[:, 1] = (np.arange(128) % 64 < R)             # sel (valid r rows)

    ctril = (norm * np.tril(np.ones((L, L), np.float64), -1)).astype(f32)

    # crows [4, 256]:
    #   [0:1, 0:128]   ones row
    #   [0:1, 128:178] -linspace(0,1,R)
    #   [0:2, 178:180] per-event STT coefficient columns
    #   [0:6, 180:255] rhs of combined A|B table matmul (block diagonal):
    #       cols 0:25  rows [g1lo^2; g1hi^2; -2g1lo; -2g1hi; 0; 0]
    #       cols 25:75 rows [0; 0; 0; 0; g2^2; -2g2]
    crows = np.zeros((6, 256), f32)
    crows[0, 0:128] = 1.0
    crows[0, 128:178] = -np.linspace(0.0, 1.0, R)
    crows[0, 178] = -beta; crows[1, 178] = 2.0 * inv2sig2    # colA -> [u; a1]
    crows[0, 179] = beta;  crows[1, 179] = 2.0 * inv2sig2    # colB -> [v; a2]
    g1lo, g1hi = g1[0:25], g1[25:50]
    crows[0, 180:205] = g1lo ** 2
    crows[1, 180:205] = g1hi ** 2
    crows[2, 180:205] = -2.0 * g1lo
    crows[3, 180:205] = -2.0 * g1hi
    crows[4, 205:255] = g2 ** 2
    crows[5, 205:255] = -2.0 * g2
    return dict(ccols=ccols, ctril=ctril, crows=crows)


def _marshal_core_inputs(t, s1, s2):
    """Pure-layout staging of one sequence's inputs (no arithmetic).

    abfast [8, 208]: critical-path operands, shipped as the first DMA:
      [0:6, 0:128]   K=6 lhsT of the combined A|B table matmul
                     [ones128; s2dup; ind_lo; ind_hi; s1lo; s1hi]
      [0:6, 128:203] rhs of the A|B matmul (block diagonal, constant)
      (row 0 of the lhsT block doubles as the all-ones broadcast row)
    icols [128, 3]: t/s1/s2 duplicated into both partition halves.
    irows [8, 704]:
      [0:1, 0:192]   t | s1 | s2 concatenated rows
      [0:4, 192:320] lhsT of the A-table matmul
                     [ind_lo; ind_hi; s1*ind_lo; s1*ind_hi]
      [0:2, 320:384] [t; s1]   (pair-packed per-event row inputs)
      [0:2, 384:448] [t; s2]
      [0:2, 448:512] [ones; s1] (rhs of per-event matmul 1)
      [0:2, 512:576] [ones; s2] (lhsT of per-event matmul 2)
      [0:6, 192:320] K=6 lhsT of the combined A|B table matmul
                     [ind_lo; ind_hi; s1*ind_lo; s1*ind_hi; ones128; s2_dup]
    """
    f32 = np.float32
    L = t.shape[0]
    icols = np.zeros((128, 3), f32)
    icols[0:L, 0] = t; icols[64:64 + L, 0] = t
    icols[0:L, 1] = s1; icols[64:64 + L, 1] = s1
    icols[0:L, 2] = s2; icols[64:64 + L, 2] = s2
    irows = np.zeros((8, 704), f32)
    irows[0, 0:L] = t
    irows[0, L:2 * L] = s1
    irows[0, 2 * L:3 * L] = s2
    irows[0, 192:256] = 1.0                       # ind_lo
    irows[1, 256:320] = 1.0                       # ind_hi
    irows[2, 192:192 + L] = s1
    irows[3, 256:256 + L] = s1
    irows[0, 320:320 + L] = t;   irows[1, 320:320 + L] = s1
    irows[0, 384:384 + L] = t;   irows[1, 384:384 + L] = s2
    irows[0, 448:448 + L] = 1.0; irows[1, 448:448 + L] = s1
    irows[0, 512:512 + L] = 1.0; irows[1, 512:512 + L] = s2
    irows[4, 192:320] = 1.0
    irows[5, 192:192 + L] = s2;  irows[5, 256:256 + L] = s2

    g1 = np.linspace(0.0, 1.0, R).astype(f32)
    g2 = np.linspace(0.0, 1.0, R).astype(f32)
    g1lo, g1hi = g1[0:25], g1[25:50]
    abfast = np.zeros((8, 208), f32)
    abfast[0, 0:128] = irows[4, 192:320]          # ones128
    abfast[1, 0:128] = irows[5, 192:320]          # s2 dup
    abfast[2:6, 0:128] = irows[0:4, 192:320]      # ind_lo; ind_hi; s1lo; s1hi
    abfast[0, 153:203] = g2 ** 2
    abfast[1, 153:203] = -2.0 * g2
    abfast[2, 128:153] = g1lo ** 2
    abfast[3, 128:153] = g1hi ** 2
    abfast[4, 128:153] = -2.0 * g1lo
    abfast[5, 128:153] = -2.0 * g1hi
    return {"icols": icols, "irows": irows, "abfast": abfast}


def _patched_act_tables(orig_fn, preferred="natural_log_exp_and_others"):
    """Wrap get_activation_tables so every function present in the
    preferred set resolves only to it (same names/order, so the emitted
    act_func_set_id still indexes the real act_info.json)."""
    import functools

    @functools.cache
    def wrapper(arch):
        tables = dict(orig_fn(arch))
        pref = tables.get(preferred)
        if not pref:
            return tables
        return {
            name: (funcs if name == preferred else funcs - pref)
            for name, funcs in tables.items()
        }
    return wrapper


def _build_program(mu: float, beta: float, inv2sig2: float, norm: float, L: int):
    import concourse.bass as bass
    import concourse.bacc as bacc
    import concourse.tile as tile
    import concourse.mybir as mybir

    f32 = mybir.dt.float32
    bf16 = mybir.dt.bfloat16
    Act = mybir.ActivationFunctionType
    Op = mybir.AluOpType

    nc = bacc.Bacc("TRN2", target_bir_lowering=False, debug=False,
                   enable_asserts=True, num_devices=NCORES)

    # ---- DRAM I/O
    abfast_d = nc.dram_tensor("abfast", [8, 208], f32, kind="ExternalInput").ap()
    icols_d = nc.dram_tensor("icols", [128, 3], f32, kind="ExternalInput").ap()
    irows_d = nc.dram_tensor("irows", [8, 704], f32, kind="ExternalInput").ap()
    ccols_d = nc.dram_tensor("ccols", [128, 2], f32, kind="ExternalInput").ap()
    ctril_d = nc.dram_tensor("ctril", [L, L], f32, kind="ExternalInput").ap()
    crows_d = nc.dram_tensor("crows", [6, 256], f32, kind="ExternalInput").ap()
    lams_o = nc.dram_tensor("lams_o", [L], f32, kind="ExternalOutput").ap()
    ll_o = nc.dram_tensor("ll_o", [1], f32, kind="ExternalOutput").ap()

    with tile.TileContext(nc) as tc, ExitStack() as ctx:
        pool = ctx.enter_context(tc.tile_pool(name="sbuf", bufs=1))
        cpool = ctx.enter_context(tc.tile_pool(name="chunk", bufs=3))
        psum = ctx.enter_context(tc.tile_pool(name="psum", bufs=1,
                                              space=bass.MemorySpace.PSUM))
        psmall = ctx.enter_context(tc.tile_pool(name="psmall", bufs=3,
                                                space=bass.MemorySpace.PSUM))
        pchunk = ctx.enter_context(tc.tile_pool(name="pchunk", bufs=2,
                                                space=bass.MemorySpace.PSUM))

        # ---- loads (6 small DMAs on 3 queues; abfast first: critical path)
        abfast = pool.tile([8, 208], f32)
        nc.sync.dma_start(abfast[:], abfast_d[:])
        icols = pool.tile([128, 3], f32)
        nc.sync.dma_start(icols[:], icols_d[:])
        irows = pool.tile([8, 704], f32)
        nc.gpsimd.dma_start(irows[:], irows_d[:])
        crows = pool.tile([6, 256], f32)
        nc.gpsimd.dma_start(crows[:], crows_d[:])
        ccols = pool.tile([128, 2], f32)
        nc.scalar.dma_start(ccols[:], ccols_d[:])
        ctril = pool.tile([L, L], f32)
        nc.scalar.dma_start(ctril[:], ctril_d[:])

        t_col = icols[:, 0:1]
        s1_col = icols[:, 1:2]
        s2_col = icols[:, 2:3]
        t_row = irows[0:1, 0:L]
        s1_row = irows[0:1, L:2 * L]
        s2_row = irows[0:1, 2 * L:3 * L]
        lhsT6AB = abfast[0:6, 0:128]
        pA_in = irows[0:2, 320:320 + L]
        pB_in = irows[0:2, 384:384 + L]
        rhs_mm1 = irows[0:2, 448:448 + L]
        lhsT_mm2 = irows[0:2, 512:512 + L]

        ones_col = ccols[:, 0:1]
        sel_col = ccols[:, 1:2]
        ones_r = abfast[0:1, 0:128]
        negttg = crows[0:1, 128:178]
        scoefA = crows[0:2, 178:179]
        scoefB = crows[0:2, 179:180]
        rhsAB = abfast[0:6, 128:203]

        negmu_col = pool.tile([128, 1], f32)
        nc.vector.memset(negmu_col[:], -mu)

        # ---- per-partition spatial biases: -inv2sig2*s1^2, -inv2sig2*s2^2
        biasA = pool.tile([128, 1], f32)
        nc.vector.tensor_scalar(biasA[:], s1_col, s1_col, -inv2sig2,
                                Op.mult, Op.mult)
        biasB = pool.tile([128, 1], f32)
        nc.vector.tensor_scalar(biasB[:], s2_col, s2_col, -inv2sig2,
                                Op.mult, Op.mult)

        # ---- separable spatial kernel: G[p, i*50+k] = A[p, i] * B[p, k]
        psAB = psmall.tile([128, 80], f32, tag="small")
        nc.tensor.matmul(psAB[:, 0:75], lhsT6AB, rhsAB, start=True, stop=True)
        A2 = pool.tile([128, 25], f32)
        nc.scalar.activation(A2[:], psAB[:, 0:25], Act.Exp,
                             scale=-inv2sig2, bias=biasA[:, 0:1])
        B2 = pool.tile([128, R], f32)
        nc.scalar.activation(B2[:], psAB[:, 25:75], Act.Exp,
                             scale=-inv2sig2, bias=biasB[:, 0:1])
        G = pool.tile([128, HALF], bf16)

        def _emit_g(goff, gw):
            a0 = goff // R
            nc.vector.tensor_tensor(
                G[:, goff:goff + gw].rearrange("p (a b) -> p a b", a=gw // R),
                A2[:, a0:a0 + gw // R].unsqueeze(2).broadcast_to(
                    [128, gw // R, R]),
                B2[:].unsqueeze(1).broadcast_to([128, gw // R, R]),
                Op.mult)

        _emit_g(0, 600)

        # ---- temporal weights W_T[j(packed), r] (norm folded into mask)
        bc_ps = psmall.tile([128, R], f32, tag="small")
        nc.tensor.matmul(bc_ps[:], ones_r, negttg, start=True, stop=True)
        dtW = pool.tile([128, R], f32)
        nc.vector.tensor_scalar(dtW[:], bc_ps[:], t_col, None, Op.add)
        Ew = pool.tile([128, R], f32)
        nc.scalar.activation(Ew[:], dtW[:], Act.Exp, scale=beta)
        hn_col = pool.tile([128, 1], f32)
        nc.vector.tensor_scalar(hn_col[:], t_col, 0.0, norm, Op.is_gt, Op.mult)
        h_col = pool.tile([128, 1], f32)
        nc.vector.tensor_scalar(h_col[:], t_col, 0.0, None, Op.is_gt)
        Mw = pool.tile([128, R], f32)
        nc.vector.tensor_scalar(Mw[:], dtW[:], 0.0, hn_col[:, 0:1], Op.is_le, Op.mult)
        WT2 = pool.tile([128, 64], bf16)
        nc.vector.memset(WT2[:], 0.0)
        nc.vector.tensor_tensor(WT2[:, 0:R], Ew[:], Mw[:], Op.mult)

        # ---- per-event exponent via 2 accumulated K=2 matmuls (fp32)
        sq1 = pool.tile([1, L], f32)
        nc.vector.tensor_tensor(sq1[:], s1_row, s1_row, Op.mult)
        sq2 = pool.tile([1, L], f32)
        nc.vector.tensor_tensor(sq2[:], s2_row, s2_row, Op.mult)
        ssum = pool.tile([1, L], f32)
        nc.vector.tensor_tensor(ssum[:], sq1[:], sq2[:], Op.add)
        w2 = pool.tile([2, L], f32)
        nc.vector.memset(w2[:], 0.0)
        nc.vector.tensor_scalar(w2[0:1, :], ssum[:], -inv2sig2, None, Op.mult)

        pairA = pool.tile([2, L], f32)
        nc.vector.scalar_tensor_tensor(pairA[:], pA_in, scoefA, w2[:],
                                       Op.mult, Op.add)
        pairB = pool.tile([2, L], f32)
        nc.vector.scalar_tensor_tensor(pairB[:], pB_in, scoefB, w2[:],
                                       Op.mult, Op.add)
        ha = pool.tile([1, L], f32)
        nc.vector.tensor_scalar(ha[:], t_row, 0.0, 1.0, Op.is_gt, Op.subtract)
        hm = pool.tile([1, L], f32)
        nc.vector.tensor_scalar(hm[:], ha[:], BIG_NEG, None, Op.mult)
        nc.vector.tensor_tensor(pairB[0:1, :], pairB[0:1, :], hm[:], Op.add)

        zev_ps = psmall.tile([L, L], f32, tag="small")
        nc.tensor.matmul(zev_ps[:], pairA[:], rhs_mm1, start=True, stop=False)
        nc.tensor.matmul(zev_ps[:], lhsT_mm2, pairB[:], start=False, stop=True)

        Ke = pool.tile([L, L], f32)
        nc.scalar.activation(Ke[:], zev_ps[:], Act.Exp)
        Km = pool.tile([L, L], f32)
        lam_col = pool.tile([L, 1], f32)
        nc.vector.scalar_tensor_tensor(Km[:], Ke[:], 0.0, ctril[:],
                                       Op.add, Op.mult, accum_out=lam_col[:])
        _emit_g(600, 650)
        # row sums of G (for the analytic relu part of softplus):
        # Gsum[p] = sum_i A2[p,i] * sum_k B2[p,k]
        sA = pool.tile([128, 1], f32)
        nc.vector.tensor_reduce(sA[:], A2[:], mybir.AxisListType.X, Op.add)
        sB = pool.tile([128, 1], f32)
        nc.vector.tensor_reduce(sB[:], B2[:], mybir.AxisListType.X, Op.add)
        gsum = pool.tile([128, 1], bf16)
        nc.vector.tensor_tensor(gsum[:], sA[:], sB[:], Op.mult)

        # lams = softplus(lam_raw + mu) + 1e-5; lam_raw >= 0 and mu >= 0, so
        # softplus(v) = v + log1p(exp(-v)) with no abs/relu needed
        ee = pool.tile([L, 1], f32)
        nc.scalar.activation(ee[:], lam_col[:], Act.Exp, scale=-1.0,
                             bias=negmu_col[0:L, 0:1])
        lp = pool.tile([L, 1], f32)
        nc.scalar.activation(lp[:], ee[:], Act.Ln, bias=1.0)
        lams_col = pool.tile([L, 1], f32)
        nc.vector.scalar_tensor_tensor(lams_col[:], lam_col[:], mu + 1e-5,
                                       lp[:], Op.add, Op.add)
        nc.sync.dma_start(lams_o.unsqueeze(1), lams_col[:])
        lnl = pool.tile([L, 1], f32)
        nc.scalar.activation(lnl[:], lams_col[:], Act.Ln)
        lnm = pool.tile([L, 1], f32)
        nc.vector.tensor_tensor(lnm[:], lnl[:], h_col[0:L, 0:1], Op.mult)

        # ---- grid: matmuls per 512 PSUM bank; softplus pipelined in two
        #      625-wide chunks (bank-level deps let them overlap the mms).
        # v = z + mu >= 0 always (z is a sum of nonnegative products and
        # mu >= 0), so softplus(v) = v + log1p(exp(-v)); the linear part
        # is summed analytically via Gsum below, only log1p runs per-element.
        # acc cols 0:3 = per-chunk log1p row sums
        acc = pool.tile([128, 3], f32)
        zz = pool.tile([128, 512], f32)
        nc.vector.memset(zz[:], 0.0)
        z_ps = psum.tile([128, 1536], f32, tag="z")
        SP_CHUNKS = CHUNKS
        emitted = set()

        def _mms_for(lo, hi):
            for moff, mw in CHUNKS:
                if moff in emitted or moff >= hi or moff + mw <= lo:
                    continue
                emitted.add(moff)
                for h in (0, 1):
                    p0 = h * 64
                    nc.tensor.matmul(z_ps[p0:p0 + 64, moff:moff + mw],
                                     WT2[p0:p0 + 64, 0:64],
                                     G[p0:p0 + 64, moff:moff + mw],
                                     start=True, stop=True)

        for ci, (off, w) in enumerate(SP_CHUNKS):
            _mms_for(off, off + w)
            e_c = pchunk.tile([128, 512], f32, tag="e_c")
            nc.scalar.activation(e_c[:, 0:w], z_ps[:, off:off + w], Act.Exp,
                                 scale=-1.0, bias=negmu_col[:, 0:1])
            l_c = cpool.tile([128, 640], f32, tag="l_c")
            nc.scalar.activation(l_c[:, 0:w], e_c[:, 0:w], Act.Ln, bias=1.0)
            s_c = cpool.tile([128, 640], f32, tag="s_c")
            nc.vector.scalar_tensor_tensor(s_c[:, 0:w], l_c[:, 0:w], 0.0,
                                           zz[:, 0:w], Op.add, Op.add,
                                           accum_out=acc[:, ci:ci + 1])

        # ---- reductions and final combine
        # zsum[r] = sum_g z[r, g] via Gsum (both packed halves accumulate)
        zs_ps = psmall.tile([64, 1], f32, tag="small")
        nc.tensor.matmul(zs_ps[:], WT2[0:64, 0:64], gsum[0:64, 0:1],
                         start=True, stop=False)
        nc.tensor.matmul(zs_ps[:], WT2[64:128, 0:64], gsum[64:128, 0:1],
                         start=False, stop=True)
        zc = pool.tile([64, 1], f32)
        nc.vector.tensor_copy(zc[:], zs_ps[:])

        red_ps = psmall.tile([1, 8], f32, tag="small")
        nc.tensor.matmul(red_ps[0:1, 0:1], lnm[:, 0:1], ones_col[0:L, 0:1],
                         start=True, stop=True)
        sl_sb = pool.tile([1, 1], f32)
        nc.vector.tensor_scalar(sl_sb[:], red_ps[0:1, 0:1],
                                -UNIT_VOL * float(RG * R) * mu, None, Op.add)
        nc.tensor.matmul(red_ps[0:1, 1:2], zc[:, 0:1], ones_col[0:L, 0:1],
                         start=True, stop=True)
        nc.tensor.matmul(red_ps[0:1, 2:5], sel_col, acc[:, 0:3],
                         start=True, stop=True)

        # loglik = sumlog - V*(zsum_tot + 125000*mu + l1p_tot)
        ird = pool.tile([1, 1], f32)
        nc.vector.tensor_reduce(ird[:], red_ps[0:1, 1:5],
                                mybir.AxisListType.X, Op.add)
        ll_sb = pool.tile([1, 1], f32)
        nc.vector.scalar_tensor_tensor(ll_sb[:], ird[:], -UNIT_VOL,
                                       sl_sb[:], Op.mult, Op.add)
        nc.sync.dma_start(ll_o.unsqueeze(1), ll_sb[:])

    import concourse.hw_specs as hw_specs
    orig = bacc.get_activation_tables
    bacc.get_activation_tables = _patched_act_tables(hw_specs.get_activation_tables)
    try:
        nc.compile()
    finally:
        bacc.get_activation_tables = orig
    return nc


def _get_program(mu, alpha, beta, sigma, L):
    key = (float(mu), float(alpha), float(beta), float(sigma), L)
    if key not in _prog_cache:
        sig2 = float(sigma) * float(sigma)
        inv2sig2 = 1.0 / (2.0 * sig2)
        norm = float(alpha) * float(beta) / (2.0 * math.pi * sig2)
        nc = _build_program(float(mu), float(beta), inv2sig2, norm, L)
        consts = _const_arrays(L, norm, float(beta), inv2sig2)
        _prog_cache[key] = (nc, consts)
    return _prog_cache[key]


def kernel(x, mu, alpha, beta, sigma):
    from concourse.bass_utils import run_bass_kernel_spmd

    x = np.asarray(x, dtype=np.float32)
    B, L, _ = x.shape
    assert B == NCORES, f"expected batch {NCORES}, got {B}"

    nc, consts = _get_program(mu, alpha, beta, sigma, L)

    in_maps = []
    for b in range(B):
        m = dict(consts)
        m.update(_marshal_core_inputs(x[b, :, 0], x[b, :, 1], x[b, :, 2]))
        in_maps.append(m)

    res = run_bass_kernel_spmd(nc, in_maps, list(range(NCORES)))
    lams = np.stack([res.results[b]["lams_o"] for b in range(B)]).astype(np.float32)
    loglik = np.stack([res.results[b]["ll_o"][0] for b in range(B)]).astype(np.float32)
    return lams, loglik
